# revision 1
# baseline (speedup 1.0000x reference)
"""Trainium2 Bass kernel for nn_Decoder_21062519620139.

6-layer post-LN transformer decoder: B=8, L=1024, D=512, H=8 heads (dk=64),
FFN 512->64->512, causal mask, sinusoidal positional encoding, embedding MLP.

Sharding: pure data-parallel over batch B=8 across the 8 NeuronCores (one
batch element per core, zero collectives).

On-device layout: all activations are feature-major ("fm": feature on the
partition axis, sequence on the free axis) so every linear layer is
matmul(out_fm, lhsT=W, rhs=x_fm) with no transposes.  Attention scores are
computed transposed (k on partitions, q on free) so softmax normalization
folds into the matmuls: exp() is a single ACT pass, the denominator comes
from ones-columns appended to V (rows 64:128 of the AV psum hold the
replicated denominator), and the 1/denom multiply fuses into the psum->sbuf
evict.  LayerNorm (over features = partitions) uses scaled-ones matmuls on
the PE to produce mean/E[x^2] replicated across all partitions.
"""

import math
import os
import sys

import numpy as np

for _p in ("/opt/trn_rl_repo", "/root/.axon_site/_ro/trn_rl_repo"):
    if os.path.isdir(_p) and _p not in sys.path:
        sys.path.insert(0, _p)

import concourse.bass as bass
import concourse.mybir as mybir
from concourse.tile import TileContext

B, L, D_IN, D, H, NL, FF = 8, 1024, 32, 512, 8, 6, 64
DK = D // H          # 64
P = 128
S = D // P           # 4 d-chunks
LC = 2               # l-chunks of 512
KT = L // P          # 8 k-tiles
NEG = -30000.0       # causal mask additive value (exp(scale*NEG) == 0 in fp32)
EPS = 1e-5
SCALE = 1.0 / math.sqrt(DK)

F32 = mybir.dt.float32
BF16 = mybir.dt.bfloat16
ATT_DT = BF16    # dtype of the q/k/v/attn matmul operands
AF = mybir.ActivationFunctionType
ALU = mybir.AluOpType


def _p0(h):
    return 64 * (h & 1)


def _s(h):
    return h >> 1


def split_excess_waits(nc, max_waits=1):
    """walrus in this toolchain rejects >1 sem-wait per instruction
    (setupSyncWait: "Too many sync wait commands").  Hoist excess waits onto
    NoOps inserted just before, on the same engine: program order on the
    engine makes this semantically identical."""
    for f in nc.m.functions:
        for bb in f.blocks:
            new_insts = []
            for inst in bb.instructions:
                si = inst.sync_info
                if si is not None and si.on_wait and len(si.on_wait) > max_waits:
                    waits = list(si.on_wait)
                    head, tail = waits[:-max_waits], waits[-max_waits:]
                    for i, w in enumerate(head):
                        new_insts.append(mybir.InstNoOp(
                            name=f"{inst.name}-ws{i}", engine=inst.engine,
                            ins=[], outs=[],
                            sync_info=mybir.SyncInfo(on_wait=[w], on_update=[])))
                    si.on_wait = tail
                new_insts.append(inst)
            bb.instructions = new_insts


def _diag_ap(tile_ap, kt_locals, c0s):
    """AP over the diagonal 128x128 squares of an S psum tile [128, n, 512]:
    element (p, j, c) -> tile[p, kt_locals[j], c0s[j] + c].  Requires the
    (kt_local, c0) pairs to advance with a constant stride, which they do
    (kt_local +1, c0 +128 => stride 512+128=640)."""
    base = tile_ap[:]
    n = len(kt_locals)
    off = base.offset + kt_locals[0] * 512 + c0s[0]
    if n == 1:
        return bass.AP(base.tensor, off, [base.ap[0], [1, 128]])
    return bass.AP(base.tensor, off, [base.ap[0], [640, n], [1, 128]])


def _bcast_free(ap2d, n):
    """[128, 512] AP -> [128, n, 512] with stride-0 middle dim."""
    a = ap2d[:, None, :]
    return a.to_broadcast((a.shape[0], n, a.shape[2]))


def build_bass():
    nc = bass.Bass("TRN2", target_bir_lowering=False, debug=False, num_devices=8)

    # ---- DRAM I/O ----
    d_xT = nc.declare_dram_parameter("xT", [D_IN, L], F32, isOutput=False)
    d_peT = nc.declare_dram_parameter("peT", [D, L], F32, isOutput=False)
    d_cmask = nc.declare_dram_parameter("cmask", [P, P], ATT_DT, isOutput=False)
    d_ident = nc.declare_dram_parameter("ident", [P, P], ATT_DT, isOutput=False)
    d_ew1 = nc.declare_dram_parameter("emb_w1", [D_IN, D], F32, isOutput=False)
    d_eb1 = nc.declare_dram_parameter("emb_b1", [D], F32, isOutput=False)
    d_eg = nc.declare_dram_parameter("emb_g", [D], F32, isOutput=False)
    d_ebeta = nc.declare_dram_parameter("emb_beta", [D], F32, isOutput=False)
    d_ew2 = nc.declare_dram_parameter("emb_w2", [D, D], F32, isOutput=False)
    d_eb2 = nc.declare_dram_parameter("emb_b2", [D], F32, isOutput=False)
    d_wq = nc.declare_dram_parameter("wq", [NL, D, D], F32, isOutput=False)
    d_wk = nc.declare_dram_parameter("wk", [NL, D, D], F32, isOutput=False)
    d_wv = nc.declare_dram_parameter("wv", [NL, D, D], F32, isOutput=False)
    d_wo = nc.declare_dram_parameter("wo", [NL, D, D], F32, isOutput=False)
    d_bo = nc.declare_dram_parameter("bo", [NL, D], F32, isOutput=False)
    d_fw1 = nc.declare_dram_parameter("fw1", [NL, D, FF], F32, isOutput=False)
    d_fb1 = nc.declare_dram_parameter("fb1", [NL, FF], F32, isOutput=False)
    d_fw2 = nc.declare_dram_parameter("fw2", [NL, FF, D], F32, isOutput=False)
    d_fb2 = nc.declare_dram_parameter("fb2", [NL, D], F32, isOutput=False)
    d_g1 = nc.declare_dram_parameter("g1", [NL, D], F32, isOutput=False)
    d_b1 = nc.declare_dram_parameter("b1", [NL, D], F32, isOutput=False)
    d_g2 = nc.declare_dram_parameter("g2", [NL, D], F32, isOutput=False)
    d_b2 = nc.declare_dram_parameter("b2", [NL, D], F32, isOutput=False)
    d_out = nc.declare_dram_parameter("houtT", [D, L], F32, isOutput=True)

    def fm(dram_2d):  # [D, X] dram -> [p, s, X] view
        return dram_2d.rearrange("(s p) x -> p s x", p=P)

    def col(dram_1d):  # [D] dram -> [p, s] view
        return dram_1d.rearrange("(s p) -> p s", p=P)

    with TileContext(nc) as tc:
        with (
            tc.tile_pool(name="const", bufs=1) as constp,
            tc.tile_pool(name="persist", bufs=1) as persist,
            tc.tile_pool(name="act", bufs=1) as actp,
            tc.tile_pool(name="attn", bufs=1) as attnp,
            tc.tile_pool(name="w", bufs=1) as wp,
            tc.tile_pool(name="lnbig", bufs=1) as lnbig,
            tc.tile_pool(name="small", bufs=2) as smallp,
            tc.tile_pool(name="psmm", bufs=4, space="PSUM") as psmm,
            tc.tile_pool(name="pss", bufs=2, space="PSUM") as pss,
        ):
            # ---- constants ----
            cmask = constp.tile([P, P], ATT_DT, tag="cmask")
            nc.sync.dma_start(cmask[:], d_cmask.ap())
            ident = constp.tile([P, P], ATT_DT, tag="ident")
            nc.sync.dma_start(ident[:], d_ident.ap())
            ones_inv = constp.tile([P, P], F32, tag="ones_inv")
            nc.gpsimd.memset(ones_inv[:], 1.0 / D)
            epsb = constp.tile([P, 1], F32, tag="epsb")
            nc.gpsimd.memset(epsb[:], EPS)

            hT = persist.tile([P, S, L], F32, tag="hT")
            v_aug = persist.tile([P, KT, H, P], ATT_DT, tag="v_aug")
            # ones columns for the softmax denominator (rows 64:128 of AV out)
            nc.gpsimd.memset(
                bass.AP(v_aug[:].tensor, v_aug[:].offset + 64,
                        [v_aug[:].ap[0], [128, KT * H], [1, 64]]),
                1.0)

            def mm_tile():
                return psmm.tile([P, 512], F32, tag="ps", name="ps")

            def emit_ln(x, out, g_col, b_col):
                """out = LN(x) over features (partitions);
                x, out: [P, S, L] sbuf; g_col/b_col: [P, S] sbuf.
                mean/var are computed replicated across all partitions via
                scaled-ones matmuls; rstd = exp(-0.5*log(var+eps)) keeps ACT
                inside the natural_log_exp table set (no sqrt -> no table
                thrash with the attention exp)."""
                for lc in range(LC):
                    lsl = slice(lc * 512, lc * 512 + 512)
                    mean_ps = mm_tile()
                    for s in range(S):
                        nc.tensor.matmul(mean_ps[:], ones_inv[:], x[:, s, lsl],
                                         start=(s == 0), stop=(s == S - 1))
                    u = lnbig.tile([P, S, 512], F32, tag="lnu")
                    nc.vector.tensor_tensor(u[:], x[:, :, lsl],
                                            _bcast_free(mean_ps, S),
                                            ALU.subtract)
                    usq = lnbig.tile([P, S, 512], F32, tag="lnsq")
                    nc.gpsimd.tensor_tensor(usq[:], u[:], u[:], ALU.mult)
                    var_ps = mm_tile()
                    for s in range(S):
                        nc.tensor.matmul(var_ps[:], ones_inv[:], usq[:, s, :],
                                         start=(s == 0), stop=(s == S - 1))
                    lnv = smallp.tile([P, 512], F32, tag="var")
                    nc.scalar.activation(lnv[:], var_ps[:], AF.Ln, bias=epsb[:])
                    rstd = smallp.tile([P, 512], F32, tag="rstd")
                    nc.scalar.activation(rstd[:], lnv[:], AF.Exp, scale=-0.5)
                    w = lnbig.tile([P, S, 512], F32, tag="lnsq")
                    nc.vector.tensor_tensor(w[:], u[:], _bcast_free(rstd, S),
                                            ALU.mult)
                    for s in range(S):
                        nc.vector.tensor_scalar(
                            out[:, s, lsl], w[:, s, :],
                            g_col[:, s:s + 1], b_col[:, s:s + 1],
                            ALU.mult, ALU.add)

            # ================= embedding =================
            x_fm = smallp.tile([D_IN, L], F32, tag="f1")
            nc.sync.dma_start(x_fm[:], d_xT.ap())
            ew1 = smallp.tile([D_IN, D], F32, tag="evtmp")
            nc.sync.dma_start(ew1[:], d_ew1.ap())
            eb1c = smallp.tile([P, S], F32, tag="eb1c")
            nc.sync.dma_start(eb1c[:], col(d_eb1.ap()))
            egc = smallp.tile([P, S], F32, tag="egc")
            nc.sync.dma_start(egc[:], col(d_eg.ap()))
            ebetac = smallp.tile([P, S], F32, tag="ebetac")
            nc.sync.dma_start(ebetac[:], col(d_ebeta.ap()))
            ew2 = wp.tile([P, S, D], F32, tag="wq")  # reuse wq slot
            nc.sync.dma_start(ew2[:], fm(d_ew2.ap()))
            eb2c = smallp.tile([P, S], F32, tag="eb2c")
            nc.sync.dma_start(eb2c[:], col(d_eb2.ap()))

            a1 = actp.tile([P, S, L], F32, tag="qT")     # relu(x@W1+b1), fm
            for mt in range(S):
                for lc in range(LC):
                    ps = mm_tile()
                    nc.tensor.matmul(ps[:], ew1[:, mt * P:(mt + 1) * P],
                                     x_fm[:, lc * 512:(lc + 1) * 512],
                                     start=True, stop=True)
                    nc.scalar.activation(a1[:, mt, lc * 512:(lc + 1) * 512],
                                         ps[:], AF.Relu,
                                         bias=eb1c[:, mt:mt + 1])
            ln0 = actp.tile([P, S, L], F32, tag="kT")
            emit_ln(a1, ln0, egc, ebetac)
            # h = ln0 @ W2 + b2 + pe
            for mt in range(S):
                for lc in range(LC):
                    ps = mm_tile()
                    for s in range(S):
                        nc.tensor.matmul(ps[:], ew2[:, s, mt * P:(mt + 1) * P],
                                         ln0[:, s, lc * 512:(lc + 1) * 512],
                                         start=(s == 0), stop=(s == S - 1))
                    tmp = smallp.tile([P, 512], F32, tag="evtmp")
                    nc.scalar.activation(tmp[:], ps[:], AF.Identity,
                                         bias=eb2c[:, mt:mt + 1])
                    pe_t = smallp.tile([P, 512], F32, tag="evtmp")
                    nc.sync.dma_start(pe_t[:],
                                      fm(d_peT.ap())[:, mt,
                                                     lc * 512:(lc + 1) * 512])
                    nc.vector.tensor_tensor(hT[:, mt, lc * 512:(lc + 1) * 512],
                                            tmp[:], pe_t[:], ALU.add)

            # ================= layers =================
            for li in range(NL):
                wq = wp.tile([P, S, D], F32, tag="wq")
                nc.sync.dma_start(wq[:], fm(d_wq.ap()[li]))
                wk = wp.tile([P, S, D], F32, tag="wk")
                nc.sync.dma_start(wk[:], fm(d_wk.ap()[li]))
                wv = wp.tile([P, S, D], F32, tag="wv")
                nc.sync.dma_start(wv[:], fm(d_wv.ap()[li]))
                wo = wp.tile([P, S, D], F32, tag="wo")
                nc.sync.dma_start(wo[:], fm(d_wo.ap()[li]))
                fw1 = wp.tile([P, S, FF], F32, tag="fw1")
                nc.sync.dma_start(fw1[:], fm(d_fw1.ap()[li]))
                fw2 = wp.tile([FF, D], F32, tag="fw2")
                nc.sync.dma_start(fw2[:], d_fw2.ap()[li])
                boc = wp.tile([P, S], F32, tag="boc")
                nc.sync.dma_start(boc[:], col(d_bo.ap()[li]))
                fb1c = wp.tile([FF, 1], F32, tag="fb1c")
                nc.sync.dma_start(fb1c[:], d_fb1.ap()[li][:, None])
                fb2c = wp.tile([P, S], F32, tag="fb2c")
                nc.sync.dma_start(fb2c[:], col(d_fb2.ap()[li]))
                g1c = wp.tile([P, S], F32, tag="g1c")
                nc.sync.dma_start(g1c[:], col(d_g1.ap()[li]))
                b1c = wp.tile([P, S], F32, tag="b1c")
                nc.sync.dma_start(b1c[:], col(d_b1.ap()[li]))
                g2c = wp.tile([P, S], F32, tag="g2c")
                nc.sync.dma_start(g2c[:], col(d_g2.ap()[li]))
                b2c = wp.tile([P, S], F32, tag="b2c")
                nc.sync.dma_start(b2c[:], col(d_b2.ap()[li]))

                # ---- q, k projections (fm) ----
                qT = actp.tile([P, S, L], ATT_DT, tag="qTb")
                kT = actp.tile([P, S, L], ATT_DT, tag="kTb")
                for dst, wmat, eng in ((qT, wq, "act"), (kT, wk, "dve")):
                    for mt in range(S):
                        for lc in range(LC):
                            ps = mm_tile()
                            for s in range(S):
                                nc.tensor.matmul(
                                    ps[:], wmat[:, s, mt * P:(mt + 1) * P],
                                    hT[:, s, lc * 512:(lc + 1) * 512],
                                    start=(s == 0), stop=(s == S - 1))
                            nc.vector.tensor_copy(
                                dst[:, mt, lc * 512:(lc + 1) * 512], ps[:])

                # ---- v projection (lm, strided into v_aug) ----
                for kt in range(KT):
                    ps = mm_tile()
                    for s in range(S):
                        nc.tensor.matmul(ps[:],
                                         hT[:, s, kt * P:(kt + 1) * P],
                                         wv[:, s, :],
                                         start=(s == 0), stop=(s == S - 1))
                    dst = bass.AP(v_aug[:].tensor,
                                  v_aug[:].offset + kt * (H * P),
                                  [v_aug[:].ap[0], [P, H], [1, DK]])
                    src = ps[:].rearrange("p (h d) -> p h d", h=H)
                    nc.vector.tensor_copy(dst, src)

                # ---- attention ----
                o_fm = actp.tile([P, S, L], F32, tag="o_fm")
                attnT = attnp.tile([P, 12, 512], ATT_DT, tag="attnT")
                for qc in range(LC):
                    for h in range(H):
                        p0, sh = _p0(h), _s(h)
                        kts = list(range(4 * (qc + 1)))
                        idx0 = 0 if qc == 0 else 4
                        for qi in range(0, len(kts), 2):
                            pair = kts[qi:qi + 2]
                            st = pss.tile([P, 2, 512], F32, tag="S")
                            c0s = [max(0, kt * P - qc * 512) for kt in pair]
                            diag = [qc * 512 <= kt * P < (qc + 1) * 512
                                    for kt in pair]
                            for j, kt in enumerate(pair):
                                nc.tensor.matmul(
                                    st[:, j, c0s[j]:512],
                                    kT[p0:p0 + DK, sh, kt * P:(kt + 1) * P],
                                    qT[p0:p0 + DK, sh,
                                       qc * 512 + c0s[j]:(qc + 1) * 512],
                                    start=True, stop=not diag[j])
                            # causal mask of the diagonal 128x128 squares:
                            # accumulate I.T @ cmask = cmask on the PE
                            for j, kt in enumerate(pair):
                                if diag[j]:
                                    nc.tensor.matmul(
                                        st[:, j, c0s[j]:c0s[j] + P],
                                        ident[:], cmask[:],
                                        start=False, stop=True,
                                        skip_group_check=True)
                            # exp; skip columns the scores never wrote when
                            # the saving exceeds the extra instruction cost
                            if c0s[0] >= 256:
                                for j in range(len(pair)):
                                    nc.scalar.activation(
                                        attnT[:, idx0 + qi + j, c0s[j]:512],
                                        st[:, j, c0s[j]:512],
                                        AF.Exp, scale=SCALE)
                            else:
                                nc.scalar.activation(
                                    attnT[:, idx0 + qi:idx0 + qi + len(pair), :],
                                    st[:, 0:len(pair), :], AF.Exp, scale=SCALE)
                        # AV
                        o_ps = mm_tile()
                        for kt in kts:
                            c0 = max(0, kt * P - qc * 512)
                            nc.tensor.matmul(
                                o_ps[:, c0:512],
                                v_aug[:, kt, h, :],
                                attnT[:, idx0 + kt, c0:512],
                                start=(kt == 0), stop=(kt == kts[-1]))
                        lnd = smallp.tile([64, 512], F32, tag="lnd")
                        nc.scalar.activation(lnd[:], o_ps[64:128, :], AF.Ln)
                        rec = smallp.tile([64, 512], F32, tag="rec")
                        nc.scalar.activation(rec[:], lnd[:], AF.Exp, scale=-1.0)
                        nc.vector.tensor_tensor(
                            o_fm[p0:p0 + DK, sh, qc * 512:(qc + 1) * 512],
                            o_ps[0:64, :], rec[:], ALU.mult)

                # ---- out-proj + residual, then LN1 ----
                r1 = actp.tile([P, S, L], F32, tag="qT")
                for mt in range(S):
                    for lc in range(LC):
                        ps = mm_tile()
                        for s in range(S):
                            nc.tensor.matmul(ps[:],
                                             wo[:, s, mt * P:(mt + 1) * P],
                                             o_fm[:, s, lc * 512:(lc + 1) * 512],
                                             start=(s == 0), stop=(s == S - 1))
                        tmp = smallp.tile([P, 512], F32, tag="evtmp")
                        nc.vector.tensor_scalar(tmp[:], ps[:],
                                                boc[:, mt:mt + 1], None,
                                                ALU.add)
                        nc.gpsimd.tensor_tensor(
                            r1[:, mt, lc * 512:(lc + 1) * 512], tmp[:],
                            hT[:, mt, lc * 512:(lc + 1) * 512], ALU.add)
                emit_ln(r1, hT, g1c, b1c)

                # ---- FFN + residual, then LN2 ----
                f1 = smallp.tile([FF, LC, 512], F32, tag="f1")
                for lc in range(LC):
                    ps = mm_tile()
                    for s in range(S):
                        nc.tensor.matmul(ps[0:FF, :], fw1[:, s, :],
                                         hT[:, s, lc * 512:(lc + 1) * 512],
                                         start=(s == 0), stop=(s == S - 1))
                    nc.scalar.activation(f1[:, lc, :], ps[0:FF, :], AF.Relu,
                                         bias=fb1c[:])
                r2 = actp.tile([P, S, L], F32, tag="kT")
                for mt in range(S):
                    for lc in range(LC):
                        ps = mm_tile()
                        nc.tensor.matmul(ps[:], fw2[:, mt * P:(mt + 1) * P],
                                         f1[:, lc, :], start=True, stop=True)
                        tmp = smallp.tile([P, 512], F32, tag="evtmp")
                        nc.vector.tensor_scalar(tmp[:], ps[:],
                                                fb2c[:, mt:mt + 1], None,
                                                ALU.add)
                        nc.gpsimd.tensor_tensor(
                            r2[:, mt, lc * 512:(lc + 1) * 512], tmp[:],
                            hT[:, mt, lc * 512:(lc + 1) * 512], ALU.add)
                emit_ln(r2, hT, g2c, b2c)

            # ---- output ----
            nc.sync.dma_start(fm(d_out.ap()), hT[:])

    split_excess_waits(nc)
    return nc


def host_inputs(x, obs_time, weights):
    """Build per-core input maps.  x: [B, L, D_IN]; obs_time: [B, L];
    weights: dict of the shared weight arrays."""
    import jax.numpy as jnp
    x = np.asarray(x, np.float32)
    obs_time = np.asarray(obs_time, np.float32)
    # pe must match the reference bit-for-bit where possible: compute it with
    # jnp on the default backend, exactly mirroring the reference lines
    # (fp32 sin of args up to B*L rad differs by ~1e-2 between libm and XLA).
    t_all = jnp.asarray(obs_time)[..., None]
    div = jnp.exp(jnp.arange(0, D, 2, dtype=jnp.float32) *
                  (-math.log(10000.0) / D))
    pe_all = np.asarray(jnp.stack([jnp.sin(t_all * div), jnp.cos(t_all * div)],
                                  axis=-1).reshape(obs_time.shape[0], L, D))
    import ml_dtypes
    att_np = ml_dtypes.bfloat16
    i = np.arange(P)
    cmask = np.where(i[:, None] > i[None, :], np.float32(NEG),
                     np.float32(0.0)).astype(att_np)
    ident = np.eye(P, dtype=att_np)
    shared = {k: np.ascontiguousarray(np.asarray(v, np.float32))
              for k, v in weights.items()}
    in_maps = []
    for b in range(B):
        m = dict(shared)
        m["xT"] = np.ascontiguousarray(x[b].T)          # [D_IN, L]
        m["peT"] = np.ascontiguousarray(pe_all[b].T)    # [D, L]
        m["cmask"] = cmask
        m["ident"] = ident
        in_maps.append(m)
    return in_maps


_NC_CACHE = {}


def get_nc():
    if "nc" not in _NC_CACHE:
        _NC_CACHE["nc"] = build_bass()
    return _NC_CACHE["nc"]


def kernel(x, mask, obs_time, emb_w1, emb_b1, emb_g, emb_beta, emb_w2, emb_b2,
           wq, wk, wv, wo, bo, fw1, fb1, fw2, fb2, g1, b1, g2, b2):
    from concourse.bass_utils import run_bass_kernel_spmd
    weights = dict(emb_w1=emb_w1, emb_b1=emb_b1, emb_g=emb_g,
                   emb_beta=emb_beta, emb_w2=emb_w2, emb_b2=emb_b2,
                   wq=wq, wk=wk, wv=wv, wo=wo, bo=bo, fw1=fw1, fb1=fb1,
                   fw2=fw2, fb2=fb2, g1=g1, b1=b1, g2=g2, b2=b2)
    in_maps = host_inputs(x, obs_time, weights)
    nc = get_nc()
    res = run_bass_kernel_spmd(nc, in_maps, list(range(B)))
    out = np.stack([res.results[b]["houtT"].T for b in range(B)], axis=0)
    return out.astype(np.float32)


if __name__ == "__main__":
    sys.path.insert(0, os.path.dirname(os.path.abspath(__file__)))
    import reference
    inputs = reference.setup_inputs()
    inputs = {k: np.asarray(v) for k, v in inputs.items()}
    expected = np.asarray(reference.reference(**inputs))
    actual = kernel(**inputs)
    err = np.linalg.norm(actual - expected) / np.linalg.norm(expected)
    print("Relative error:", err)
    print("max abs err:", np.abs(actual - expected).max())

